# revision 15
# baseline (speedup 1.0000x reference)
"""Trainium2 Bass kernel for nn_BayesianLayer (sampling, contraction-sharded).

Reference computation (per full inputs):
    sigma      = softplus(ro)                  # [IN, OUT]
    sigma_b    = softplus(ro_bias)             # [1, OUT]
    weights    = eps * sigma + mu              # [B, IN, OUT]
    bias       = eps_bias * sigma_b + mu_bias  # [B, OUT]
    out        = einsum("bi,bio->bo", x, weights) + bias

Sharding: the kernel is DMA-bound (eps alone is 256 MB), so the split
minimizes per-core HBM bytes. IN=1024 is sharded across the 8 cores
(128 contraction rows each): eps, mu, ro, x are sharded along IN — so
mu/ro are NOT replicated (vs. 8 MB/core of replicated mu+ro under batch
sharding). Each core emits a partial over its i-slice; the host sums the
8 partials. The bias term is masked onto the core owning those batch
rows (bmask one-hot block) so the host sum adds it exactly once.

Input precision: everything streams as fp16 (host-side cast — DMA
halves, PE runs at 1 cycle/row vs 4 for fp32). Error is ~5e-4
max-relative vs. the 2e-2 gate.

Layout: everything runs TRANSPOSED, out_T[o, b], so all DMA and engine
work spreads across the full 128 partitions (DMA cost scales with
per-partition bytes; a row-major [1, OUT] result row would serialize on
one partition):
  - matvec per (sample b, o-chunk oc): lhsT = (eps*sigma)[128i, 128o]
    stationary, rhs = x column [128i, 1] moving -> psum[128o, oc, b].
    The whole [OUT, B] partial accumulates in ONE psum bank.
  - xmu partial: lhsT = mu chunk [128i, 128o], rhs = xT [128i, 64] ->
    a second bank; 8 matmuls cover it, emitted at the PE queue head.
  - bias in transposed layout: comb_T[o_p, oc, b] =
    bmask[b] * (eps_bias_T[o, b] * sigma_b[o] + mu_bias[o]); operands
    host-pre-transposed, sigma_b/mu_bias as per-partition scalars of a
    fused tensor_scalar (mult+add), the b-mask via a broadcast multiply.
  - epilogue: DVE evacuates psum -> fp16 stage and adds comb_T; one
    128-partition DMA ships out_T [128, 512] fp16. The host transposes
    the partials back.

Queue/buffer discipline for the For_i steady state:
  - eps chunks stream on the sync queue (optionally alternating with the
    scalar queue, BAYES_DUAL_Q) in chunk-major host layout (full 8KB+
    contiguous per-partition lines).
  - setup loads (xT/ro/mu/bias operands) ride the otherwise-idle Pool
    queue so the next iteration's prefetch is never gated.
  - sig/ro live in a bufs=2 pool and the sigma softplus chain is the
    only late ACT work, so iteration i+1's sigma is ready mid-iteration
    i; ACT also replicates sigma CB-fold so the DVE runs ONE
    tensor_tensor per chunk.
  - the bias DVE chain is emitted AFTER the streaming loop so the
    in-order DVE queue runs prods back-to-back across the boundary; the
    out DMA issues from the DVE queue right after the epilogue add.

build_nc(repeat=N) wraps the whole body in a For_i loop — used only by
the timing harness (test.py); the graded path uses repeat=1.
"""

import contextlib
import os

import numpy as np

import concourse.bass as bass
import concourse.mybir as mybir
import concourse.tile as tile
from concourse import bacc
from concourse.bass import ts
from concourse import bass_utils

B, IN, OUT = 64, 1024, 1024
NCORES = 8
P = 128            # SBUF partitions = per-core i-slice (IN / NCORES)
BL = B // NCORES   # batch rows whose bias this core owns
CB = int(os.environ.get("BAYES_CB", "4"))  # samples per eps DMA chunk
CHUNKS = B // CB
OC = OUT // P      # 8 o-chunks of 128

F32 = mybir.dt.float32
F16 = mybir.dt.float16
AF = mybir.ActivationFunctionType

EPS_BUFS = int(os.environ.get("BAYES_EPS_BUFS", "6"))
PROD_BUFS = int(os.environ.get("BAYES_PROD_BUFS", "4"))
DUAL_Q = os.environ.get("BAYES_DUAL_Q", "1") == "1"
SIGREP = os.environ.get("BAYES_SIGREP", "1") == "1"
# timing probes (correctness-breaking, never set in the graded path)
NO_MM = os.environ.get("BAYES_NO_MM", "0") == "1"      # skip matvec matmuls
NO_PROD = os.environ.get("BAYES_NO_PROD", "0") == "1"  # matvec on raw ep


def build_nc(repeat: int = 1) -> bass.Bass:
    nc = bacc.Bacc(
        "TRN2",
        target_bir_lowering=False,
        debug=False,
        num_devices=NCORES,
    )

    xT_d = nc.dram_tensor("xT", [P, B], F16, kind="ExternalInput")
    mu_d = nc.dram_tensor("mu", [P, OUT], F16, kind="ExternalInput")
    ro_d = nc.dram_tensor("ro", [P, OUT], F16, kind="ExternalInput")
    mubT_d = nc.dram_tensor("mu_bias_T", [P, OC], F32, kind="ExternalInput")
    robT_d = nc.dram_tensor("ro_bias_T", [P, OC], F32, kind="ExternalInput")
    # chunk-major host layout: per chunk each partition's CB rows are one
    # contiguous 2*CB KB run -> full-rate DMA descriptors
    eps_d = nc.dram_tensor("eps", [CHUNKS, P, CB * OUT], F16, kind="ExternalInput")
    ebsT_d = nc.dram_tensor("eps_bias_T", [P, OC * B], F16, kind="ExternalInput")
    mskb_d = nc.dram_tensor("bmask", [1, B], F32, kind="ExternalInput")
    out_d = nc.dram_tensor("out", [P, OC * B], F16, kind="ExternalOutput")

    with tile.TileContext(nc) as tc:
        with (
            tc.tile_pool(name="const", bufs=1) as const,
            tc.tile_pool(name="sigp", bufs=2) as sigp,
            tc.tile_pool(name="stream", bufs=EPS_BUFS) as stream,
            tc.tile_pool(name="prods", bufs=PROD_BUFS) as prods,
            tc.tile_pool(name="psum_acc", bufs=2, space="PSUM") as psum_acc,
            tc.tile_pool(name="psum_misc", bufs=2, space="PSUM") as psum_misc,
        ):
          with tc.For_i(0, repeat, 1) if repeat > 1 else contextlib.nullcontext():
            # ---------- setup DMAs (Pool queue: never gates prefetch) ----------
            xT_sb = const.tile([P, B], F16, name="xT_sb")
            nc.gpsimd.dma_start(xT_sb, xT_d[:])
            ro_sb = sigp.tile([P, OUT], F16, name="ro_sb")
            nc.gpsimd.dma_start(ro_sb, ro_d[:])
            mu_sb = const.tile([P, OUT], F16, name="mu_sb")
            nc.gpsimd.dma_start(mu_sb, mu_d[:])
            ebsT = const.tile([P, OC, B], F16, name="ebsT")
            nc.gpsimd.dma_start(ebsT, ebsT_d[:].rearrange("p (c b) -> p c b", b=B))
            sbbT = const.tile([P, OC], F32, name="sbbT")
            nc.gpsimd.dma_start(sbbT, robT_d[:])
            mubT = const.tile([P, OC], F32, name="mubT")
            nc.gpsimd.dma_start(mubT, mubT_d[:])
            mskb = const.tile([P, B], F32, name="mskb")
            nc.gpsimd.dma_start(mskb, mskb_d[:].to_broadcast((P, B)))

            # ---------- ACT: sigma (double-buffered, replicated) ----------
            sig_t = sigp.tile([P, OUT], F32, name="sig_t")
            nc.scalar.activation(sig_t, ro_sb, AF.Exp)
            if SIGREP:
                sigr = sigp.tile([P, CB, OUT], F16, name="sigr")
                nc.scalar.activation(sigr[:, 0, :], sig_t, AF.Ln, bias=1.0)
                for t in range(1, CB):
                    nc.scalar.copy(sigr[:, t, :], sigr[:, 0, :])
            else:
                sig = sigp.tile([P, OUT], F16, name="sig")
                nc.scalar.activation(sig, sig_t, AF.Ln, bias=1.0)
            nc.scalar.activation(sbbT, sbbT, AF.Exp)
            nc.scalar.activation(sbbT, sbbT, AF.Ln, bias=1.0)

            # ---------- PE head: xmu partial, ACT evacuates it ----------
            xmu_ps = psum_misc.tile([P, OC, B], F32, name="xmu_ps", tag="xmu")
            for c in range(OC):
                nc.tensor.matmul(
                    xmu_ps[:, c, :], mu_sb[:, ts(c, P)], xT_sb,
                    start=True, stop=True,
                )
            xmu_sb = const.tile([P, OC, B], F32, name="xmu_sb")
            nc.scalar.copy(xmu_sb, xmu_ps)

            # ---------- streaming main loop ----------
            xps = psum_acc.tile([P, OC, B], F32, name="xps", tag="xps")
            if NO_MM:
                nc.vector.memset(xps, 0.0)
            for c in range(CHUNKS):
                ep = stream.tile([P, CB * OUT], F16, name="ep", tag="ep")
                q = nc.scalar if (DUAL_Q and c % 2) else nc.sync
                q.dma_start(ep, eps_d[c])
                prod = prods.tile([P, CB * OUT], F16, name="prod", tag="prod")
                if not NO_PROD:
                    if SIGREP:
                        nc.vector.tensor_tensor(
                            prod, ep, sigr[:].rearrange("p t o -> p (t o)"),
                            mybir.AluOpType.mult,
                        )
                    else:
                        for t in range(CB):
                            nc.vector.tensor_tensor(
                                prod[:, ts(t, OUT)], ep[:, ts(t, OUT)], sig,
                                mybir.AluOpType.mult,
                            )
                src = ep if NO_PROD else prod
                if not NO_MM:
                    for t in range(CB):
                        b = c * CB + t
                        for oc in range(OC):
                            o0 = t * OUT + oc * P
                            nc.tensor.matmul(
                                xps[:, oc, b : b + 1],
                                src[:, o0 : o0 + P],
                                xT_sb[:, b : b + 1],
                                start=True, stop=True,
                            )

            # ---------- bias chain (DVE, after all prods in queue) ----------
            combT = const.tile([P, OC, B], F32, name="combT")
            for c in range(OC):
                # comb = ebs_T * sigma_b + mu_bias (fused per-partition scalars)
                nc.vector.tensor_scalar(
                    combT[:, c, :], ebsT[:, c, :],
                    sbbT[:, c : c + 1], mubT[:, c : c + 1],
                    op0=mybir.AluOpType.mult, op1=mybir.AluOpType.add,
                )
                nc.vector.tensor_tensor(
                    combT[:, c, :], combT[:, c, :], mskb, mybir.AluOpType.mult
                )
            nc.vector.tensor_add(combT, combT, xmu_sb)
            comb16 = const.tile([P, OC, B], F16, name="comb16")
            nc.vector.tensor_copy(comb16, combT)

            # ---------- epilogue: DVE evac + add, out via DVE queue ----------
            stage = const.tile([P, OC, B], F16, name="stage")
            nc.vector.tensor_copy(stage, xps)
            nc.vector.tensor_add(stage, stage, comb16)
            nc.scalar.dma_start(out_d[:], stage[:].rearrange("p c b -> p (c b)"))

    nc.finalize()
    return nc


def _shard_inputs(inputs: dict) -> list[dict]:
    x = np.asarray(inputs["x"], dtype=np.float32)
    mu = np.asarray(inputs["mu"], dtype=np.float32)
    ro = np.asarray(inputs["ro"], dtype=np.float32)
    mub = np.asarray(inputs["mu_bias"], dtype=np.float32)
    rob = np.asarray(inputs["ro_bias"], dtype=np.float32)
    eps = np.asarray(inputs["eps"], dtype=np.float32)
    ebd = np.asarray(inputs["eps_bias"], dtype=np.float32)

    xT16 = np.ascontiguousarray(x.T.astype(np.float16))       # [IN, B]
    mu16 = mu.astype(np.float16)                              # [IN, OUT]
    ro16 = ro.astype(np.float16)                              # [IN, OUT]
    eps16 = eps.astype(np.float16)                            # [B, IN, OUT]
    # chunk-major: [CHUNKS, IN, CB*OUT]; b = c*CB + t
    eps16 = np.ascontiguousarray(
        eps16.reshape(CHUNKS, CB, IN, OUT)
        .transpose(0, 2, 1, 3)
        .reshape(CHUNKS, IN, CB * OUT)
    )
    # transposed bias operands: [P(o_p), OC(oc), ...] with o = oc*128 + o_p
    ebsT = np.ascontiguousarray(
        ebd.T.reshape(OC, P, B).transpose(1, 0, 2).reshape(P, OC * B)
    ).astype(np.float16)                                      # [128, 8*64]
    mubT = np.ascontiguousarray(mub.reshape(OC, P).T).astype(np.float32)
    robT = np.ascontiguousarray(rob.reshape(OC, P).T).astype(np.float32)

    in_maps = []
    for k in range(NCORES):
        sl = slice(k * P, (k + 1) * P)
        msk = np.zeros((1, B), dtype=np.float32)
        msk[0, k * BL : (k + 1) * BL] = 1.0
        in_maps.append(
            {
                "xT": np.ascontiguousarray(xT16[sl]),
                "mu": np.ascontiguousarray(mu16[sl]),
                "ro": np.ascontiguousarray(ro16[sl]),
                "mu_bias_T": mubT,
                "ro_bias_T": robT,
                "eps": np.ascontiguousarray(eps16[:, sl, :]),  # [CHUNKS, P, CB*OUT]
                "eps_bias_T": ebsT,
                "bmask": msk,
            }
        )
    return in_maps


def _gather(stacked: np.ndarray) -> np.ndarray:
    """[NCORES, P, OC*B] per-core transposed partials -> [B, OUT] f32."""
    a = stacked.reshape(NCORES, P, OC, B).astype(np.float32).sum(axis=0)
    # a[o_p, oc, b] -> out[b, oc*128 + o_p]
    return np.ascontiguousarray(a.transpose(2, 1, 0).reshape(B, OUT))


def run(inputs: dict, trace: bool = False):
    nc = build_nc()
    in_maps = _shard_inputs(inputs)
    res = bass_utils.run_bass_kernel_spmd(
        nc, in_maps, core_ids=list(range(NCORES)), trace=trace
    )
    out = _gather(
        np.stack([res.results[k]["out"] for k in range(NCORES)], axis=0)
    )
    return out, res


def kernel(**inputs: np.ndarray) -> np.ndarray:
    try:
        out, _ = run(inputs, trace=False)
    except Exception:
        # transient device errors (NRT_EXEC_UNIT_UNRECOVERABLE) have been
        # observed to clear on retry
        import time

        time.sleep(5.0)
        out, _ = run(inputs, trace=False)
    return out


# revision 17
# speedup vs baseline: 1.2836x; 1.2836x over previous
"""Trainium2 Bass kernel for nn_BayesianLayer (sampling, contraction-sharded).

Reference computation (per full inputs):
    sigma      = softplus(ro)                  # [IN, OUT]
    sigma_b    = softplus(ro_bias)             # [1, OUT]
    weights    = eps * sigma + mu              # [B, IN, OUT]
    bias       = eps_bias * sigma_b + mu_bias  # [B, OUT]
    out        = einsum("bi,bio->bo", x, weights) + bias

Sharding: the kernel is DMA-bound (eps alone is 256 MB), so the split
minimizes per-core HBM bytes. IN=1024 is sharded across the 8 cores
(128 contraction rows each): eps, mu, ro, x are sharded along IN — so
mu/ro are NOT replicated (vs. 8 MB/core of replicated mu+ro under batch
sharding). Each core emits a partial over its i-slice; the host sums the
8 partials. The bias term is masked onto the core owning those batch
rows (bmask one-hot block) so the host sum adds it exactly once.

Input precision: everything streams as fp16 (host-side cast — DMA
halves, PE runs at 1 cycle/row vs 4 for fp32). Error is ~5e-4
max-relative vs. the 2e-2 gate.

Layout: everything runs TRANSPOSED, out_T[o, b], so all DMA and engine
work spreads across the full 128 partitions (DMA cost scales with
per-partition bytes; a row-major [1, OUT] result row would serialize on
one partition):
  - matvec per (sample b, o-chunk oc): lhsT = (eps*sigma)[128i, 128o]
    stationary, rhs = x column [128i, 1] moving -> psum[128o, oc, b].
    The whole [OUT, B] partial accumulates in ONE psum bank.
  - xmu partial: lhsT = mu chunk [128i, 128o], rhs = xT [128i, 64] ->
    a second bank; 8 matmuls cover it, emitted at the PE queue head.
  - bias in transposed layout: comb_T[o_p, oc, b] =
    bmask[b] * (eps_bias_T[o, b] * sigma_b[o] + mu_bias[o]); operands
    host-pre-transposed, sigma_b/mu_bias as per-partition scalars of a
    fused tensor_scalar (mult+add), the b-mask via a broadcast multiply.
  - epilogue: DVE evacuates psum -> fp16 stage and adds comb_T; one
    128-partition DMA ships out_T [128, 512] fp16. The host transposes
    the partials back.

Queue/buffer discipline for the For_i steady state:
  - eps chunks stream on the sync queue (optionally alternating with the
    scalar queue, BAYES_DUAL_Q) in chunk-major host layout (full 8KB+
    contiguous per-partition lines).
  - setup loads (xT/ro/mu/bias operands) ride the otherwise-idle Pool
    queue so the next iteration's prefetch is never gated.
  - sig/ro live in a bufs=2 pool and the sigma softplus chain is the
    only late ACT work, so iteration i+1's sigma is ready mid-iteration
    i; ACT also replicates sigma CB-fold so the DVE runs ONE
    tensor_tensor per chunk.
  - the bias DVE chain is emitted AFTER the streaming loop so the
    in-order DVE queue runs prods back-to-back across the boundary; the
    out DMA issues from the DVE queue right after the epilogue add.

build_nc(repeat=N) wraps the whole body in a For_i loop — used only by
the timing harness (test.py); the graded path uses repeat=1.
"""

import contextlib
import os

import numpy as np

import concourse.bass as bass
import concourse.mybir as mybir
import concourse.tile as tile
from concourse import bacc
from concourse.bass import ts
from concourse import bass_utils

B, IN, OUT = 64, 1024, 1024
NCORES = 8
P = 128            # SBUF partitions = per-core i-slice (IN / NCORES)
BL = B // NCORES   # batch rows whose bias this core owns
CB = int(os.environ.get("BAYES_CB", "4"))  # samples per eps DMA chunk
CHUNKS = B // CB
OC = OUT // P      # 8 o-chunks of 128

F32 = mybir.dt.float32
F16 = mybir.dt.float16
AF = mybir.ActivationFunctionType

EPS_BUFS = int(os.environ.get("BAYES_EPS_BUFS", "6"))
PROD_BUFS = int(os.environ.get("BAYES_PROD_BUFS", "4"))
DUAL_Q = os.environ.get("BAYES_DUAL_Q", "0") == "1"
SIGREP = os.environ.get("BAYES_SIGREP", "1") == "1"
INPLACE = os.environ.get("BAYES_INPLACE", "0") == "1"
# timing probes (correctness-breaking, never set in the graded path)
NO_MM = os.environ.get("BAYES_NO_MM", "0") == "1"      # skip matvec matmuls
NO_PROD = os.environ.get("BAYES_NO_PROD", "0") == "1"  # matvec on raw ep


def build_nc(repeat: int = 1) -> bass.Bass:
    nc = bacc.Bacc(
        "TRN2",
        target_bir_lowering=False,
        debug=False,
        num_devices=NCORES,
    )

    xT_d = nc.dram_tensor("xT", [P, B], F16, kind="ExternalInput")
    mu_d = nc.dram_tensor("mu", [P, OUT], F16, kind="ExternalInput")
    ro_d = nc.dram_tensor("ro", [P, OUT], F16, kind="ExternalInput")
    mubT_d = nc.dram_tensor("mu_bias_T", [P, OC], F32, kind="ExternalInput")
    robT_d = nc.dram_tensor("ro_bias_T", [P, OC], F32, kind="ExternalInput")
    # chunk-major host layout: per chunk each partition's CB rows are one
    # contiguous 2*CB KB run -> full-rate DMA descriptors
    eps_d = nc.dram_tensor("eps", [CHUNKS, P, CB * OUT], F16, kind="ExternalInput")
    ebsT_d = nc.dram_tensor("eps_bias_T", [P, OC * B], F16, kind="ExternalInput")
    mskb_d = nc.dram_tensor("bmask", [1, B], F32, kind="ExternalInput")
    out_d = nc.dram_tensor("out", [P, OC * B], F16, kind="ExternalOutput")

    with tile.TileContext(nc) as tc:
        with (
            tc.tile_pool(name="const", bufs=1) as const,
            tc.tile_pool(name="sigp", bufs=2) as sigp,
            tc.tile_pool(name="stream", bufs=EPS_BUFS) as stream,
            tc.tile_pool(name="prods", bufs=PROD_BUFS) as prods,
            tc.tile_pool(name="psum_acc", bufs=2, space="PSUM") as psum_acc,
            tc.tile_pool(name="psum_misc", bufs=2, space="PSUM") as psum_misc,
        ):
          with tc.For_i(0, repeat, 1) if repeat > 1 else contextlib.nullcontext():
            # ---------- setup DMAs (Pool queue: never gates prefetch) ----------
            xT_sb = const.tile([P, B], F16, name="xT_sb")
            nc.gpsimd.dma_start(xT_sb, xT_d[:])
            ro_sb = sigp.tile([P, OUT], F16, name="ro_sb")
            nc.gpsimd.dma_start(ro_sb, ro_d[:])
            mu_sb = const.tile([P, OUT], F16, name="mu_sb")
            nc.gpsimd.dma_start(mu_sb, mu_d[:])
            ebsT = const.tile([P, OC, B], F16, name="ebsT")
            nc.gpsimd.dma_start(ebsT, ebsT_d[:].rearrange("p (c b) -> p c b", b=B))
            sbbT = const.tile([P, OC], F32, name="sbbT")
            nc.gpsimd.dma_start(sbbT, robT_d[:])
            mubT = const.tile([P, OC], F32, name="mubT")
            nc.gpsimd.dma_start(mubT, mubT_d[:])
            mskb = const.tile([P, B], F32, name="mskb")
            nc.gpsimd.dma_start(mskb, mskb_d[:].to_broadcast((P, B)))

            # ---------- ACT: sigma (double-buffered, replicated) ----------
            sig_t = sigp.tile([P, OUT], F32, name="sig_t")
            nc.scalar.activation(sig_t, ro_sb, AF.Exp)
            if SIGREP:
                sigr = sigp.tile([P, CB, OUT], F16, name="sigr")
                nc.scalar.activation(sigr[:, 0, :], sig_t, AF.Ln, bias=1.0)
                for t in range(1, CB):
                    nc.scalar.copy(sigr[:, t, :], sigr[:, 0, :])
            else:
                sig = sigp.tile([P, OUT], F16, name="sig")
                nc.scalar.activation(sig, sig_t, AF.Ln, bias=1.0)
            nc.scalar.activation(sbbT, sbbT, AF.Exp)
            nc.scalar.activation(sbbT, sbbT, AF.Ln, bias=1.0)

            # ---------- PE head: xmu partial, ACT evacuates it ----------
            xmu_ps = psum_misc.tile([P, OC, B], F32, name="xmu_ps", tag="xmu")
            for c in range(OC):
                nc.tensor.matmul(
                    xmu_ps[:, c, :], mu_sb[:, ts(c, P)], xT_sb,
                    start=True, stop=True,
                )
            xmu_sb = const.tile([P, OC, B], F32, name="xmu_sb")
            nc.scalar.copy(xmu_sb, xmu_ps)

            # ---------- streaming main loop ----------
            xps = psum_acc.tile([P, OC, B], F32, name="xps", tag="xps")
            if NO_MM:
                nc.vector.memset(xps, 0.0)
            for c in range(CHUNKS):
                ep = stream.tile([P, CB * OUT], F16, name="ep", tag="ep")
                q = nc.scalar if (DUAL_Q and c % 2) else nc.sync
                q.dma_start(ep, eps_d[c])
                if INPLACE:
                    prod = ep
                else:
                    prod = prods.tile([P, CB * OUT], F16, name="prod", tag="prod")
                if not NO_PROD:
                    if SIGREP:
                        nc.vector.tensor_tensor(
                            prod, ep, sigr[:].rearrange("p t o -> p (t o)"),
                            mybir.AluOpType.mult,
                        )
                    else:
                        for t in range(CB):
                            nc.vector.tensor_tensor(
                                prod[:, ts(t, OUT)], ep[:, ts(t, OUT)], sig,
                                mybir.AluOpType.mult,
                            )
                src = ep if NO_PROD else prod
                if not NO_MM:
                    for t in range(CB):
                        b = c * CB + t
                        for oc in range(OC):
                            o0 = t * OUT + oc * P
                            nc.tensor.matmul(
                                xps[:, oc, b : b + 1],
                                src[:, o0 : o0 + P],
                                xT_sb[:, b : b + 1],
                                start=True, stop=True,
                            )

            # ---------- bias chain (DVE, after all prods in queue) ----------
            combT = const.tile([P, OC, B], F32, name="combT")
            for c in range(OC):
                # comb = ebs_T * sigma_b + mu_bias (fused per-partition scalars)
                nc.vector.tensor_scalar(
                    combT[:, c, :], ebsT[:, c, :],
                    sbbT[:, c : c + 1], mubT[:, c : c + 1],
                    op0=mybir.AluOpType.mult, op1=mybir.AluOpType.add,
                )
                nc.vector.tensor_tensor(
                    combT[:, c, :], combT[:, c, :], mskb, mybir.AluOpType.mult
                )
            nc.vector.tensor_add(combT, combT, xmu_sb)
            comb16 = const.tile([P, OC, B], F16, name="comb16")
            nc.vector.tensor_copy(comb16, combT)

            # ---------- epilogue: DVE evac + add, out via DVE queue ----------
            stage = const.tile([P, OC, B], F16, name="stage")
            nc.vector.tensor_copy(stage, xps)
            nc.vector.tensor_add(stage, stage, comb16)
            nc.scalar.dma_start(out_d[:], stage[:].rearrange("p c b -> p (c b)"))

    nc.finalize()
    return nc


def _shard_inputs(inputs: dict) -> list[dict]:
    x = np.asarray(inputs["x"], dtype=np.float32)
    mu = np.asarray(inputs["mu"], dtype=np.float32)
    ro = np.asarray(inputs["ro"], dtype=np.float32)
    mub = np.asarray(inputs["mu_bias"], dtype=np.float32)
    rob = np.asarray(inputs["ro_bias"], dtype=np.float32)
    eps = np.asarray(inputs["eps"], dtype=np.float32)
    ebd = np.asarray(inputs["eps_bias"], dtype=np.float32)

    xT16 = np.ascontiguousarray(x.T.astype(np.float16))       # [IN, B]
    mu16 = mu.astype(np.float16)                              # [IN, OUT]
    ro16 = ro.astype(np.float16)                              # [IN, OUT]
    eps16 = eps.astype(np.float16)                            # [B, IN, OUT]
    # chunk-major: [CHUNKS, IN, CB*OUT]; b = c*CB + t
    eps16 = np.ascontiguousarray(
        eps16.reshape(CHUNKS, CB, IN, OUT)
        .transpose(0, 2, 1, 3)
        .reshape(CHUNKS, IN, CB * OUT)
    )
    # transposed bias operands: [P(o_p), OC(oc), ...] with o = oc*128 + o_p
    ebsT = np.ascontiguousarray(
        ebd.T.reshape(OC, P, B).transpose(1, 0, 2).reshape(P, OC * B)
    ).astype(np.float16)                                      # [128, 8*64]
    mubT = np.ascontiguousarray(mub.reshape(OC, P).T).astype(np.float32)
    robT = np.ascontiguousarray(rob.reshape(OC, P).T).astype(np.float32)

    in_maps = []
    for k in range(NCORES):
        sl = slice(k * P, (k + 1) * P)
        msk = np.zeros((1, B), dtype=np.float32)
        msk[0, k * BL : (k + 1) * BL] = 1.0
        in_maps.append(
            {
                "xT": np.ascontiguousarray(xT16[sl]),
                "mu": np.ascontiguousarray(mu16[sl]),
                "ro": np.ascontiguousarray(ro16[sl]),
                "mu_bias_T": mubT,
                "ro_bias_T": robT,
                "eps": np.ascontiguousarray(eps16[:, sl, :]),  # [CHUNKS, P, CB*OUT]
                "eps_bias_T": ebsT,
                "bmask": msk,
            }
        )
    return in_maps


def _gather(stacked: np.ndarray) -> np.ndarray:
    """[NCORES, P, OC*B] per-core transposed partials -> [B, OUT] f32."""
    a = stacked.reshape(NCORES, P, OC, B).astype(np.float32).sum(axis=0)
    # a[o_p, oc, b] -> out[b, oc*128 + o_p]
    return np.ascontiguousarray(a.transpose(2, 1, 0).reshape(B, OUT))


def run(inputs: dict, trace: bool = False):
    nc = build_nc()
    in_maps = _shard_inputs(inputs)
    res = bass_utils.run_bass_kernel_spmd(
        nc, in_maps, core_ids=list(range(NCORES)), trace=trace
    )
    out = _gather(
        np.stack([res.results[k]["out"] for k in range(NCORES)], axis=0)
    )
    return out, res


def kernel(**inputs: np.ndarray) -> np.ndarray:
    try:
        out, _ = run(inputs, trace=False)
    except Exception:
        # transient device errors (NRT_EXEC_UNIT_UNRECOVERABLE) have been
        # observed to clear on retry
        import time

        time.sleep(5.0)
        out, _ = run(inputs, trace=False)
    return out


# revision 18
# speedup vs baseline: 1.3245x; 1.0319x over previous
"""Trainium2 Bass kernel for nn_BayesianLayer (sampling, contraction-sharded).

Reference computation (per full inputs):
    sigma      = softplus(ro)                  # [IN, OUT]
    sigma_b    = softplus(ro_bias)             # [1, OUT]
    weights    = eps * sigma + mu              # [B, IN, OUT]
    bias       = eps_bias * sigma_b + mu_bias  # [B, OUT]
    out        = einsum("bi,bio->bo", x, weights) + bias

Sharding: the kernel is DMA-bound (eps alone is 256 MB), so the split
minimizes per-core HBM bytes. IN=1024 is sharded across the 8 cores
(128 contraction rows each): eps, mu, ro, x are sharded along IN — so
mu/ro are NOT replicated (vs. 8 MB/core of replicated mu+ro under batch
sharding). Each core emits a partial over its i-slice; the host sums the
8 partials. The bias term is masked onto the core owning those batch
rows (bmask one-hot block) so the host sum adds it exactly once.

Input precision: everything streams as fp16 (host-side cast — DMA
halves, PE runs at 1 cycle/row vs 4 for fp32). Error is ~5e-4
max-relative vs. the 2e-2 gate.

Layout: everything runs TRANSPOSED, out_T[o, b], so all DMA and engine
work spreads across the full 128 partitions (DMA cost scales with
per-partition bytes; a row-major [1, OUT] result row would serialize on
one partition):
  - matvec per (sample b, o-chunk oc): lhsT = (eps*sigma)[128i, 128o]
    stationary, rhs = x column [128i, 1] moving -> psum[128o, oc, b].
    The whole [OUT, B] partial accumulates in ONE psum bank.
  - xmu partial: lhsT = mu chunk [128i, 128o], rhs = xT [128i, 64] ->
    a second bank; 8 matmuls cover it, emitted at the PE queue head.
  - bias in transposed layout: comb_T[o_p, oc, b] =
    bmask[b] * (eps_bias_T[o, b] * sigma_b[o] + mu_bias[o]); operands
    host-pre-transposed, sigma_b/mu_bias as per-partition scalars of a
    fused tensor_scalar (mult+add), the b-mask via a broadcast multiply.
  - epilogue: DVE evacuates psum -> fp16 stage and adds comb_T; one
    128-partition DMA ships out_T [128, 512] fp16. The host transposes
    the partials back.

Queue/buffer discipline for the For_i steady state:
  - eps chunks stream on the sync queue (optionally alternating with the
    scalar queue, BAYES_DUAL_Q) in chunk-major host layout (full 8KB+
    contiguous per-partition lines).
  - setup loads (xT/ro/mu/bias operands) ride the otherwise-idle Pool
    queue so the next iteration's prefetch is never gated.
  - sig/ro live in a bufs=2 pool and the sigma softplus chain is the
    only late ACT work, so iteration i+1's sigma is ready mid-iteration
    i; ACT also replicates sigma CB-fold so the DVE runs ONE
    tensor_tensor per chunk.
  - the bias DVE chain is emitted AFTER the streaming loop so the
    in-order DVE queue runs prods back-to-back across the boundary; the
    out DMA issues from the DVE queue right after the epilogue add.

build_nc(repeat=N) wraps the whole body in a For_i loop — used only by
the timing harness (test.py); the graded path uses repeat=1.
"""

import contextlib
import os

import numpy as np

import concourse.bass as bass
import concourse.mybir as mybir
import concourse.tile as tile
from concourse import bacc
from concourse.bass import ts
from concourse import bass_utils

B, IN, OUT = 64, 1024, 1024
NCORES = 8
P = 128            # SBUF partitions = per-core i-slice (IN / NCORES)
BL = B // NCORES   # batch rows whose bias this core owns
CB = int(os.environ.get("BAYES_CB", "4"))  # samples per eps DMA chunk
CHUNKS = B // CB
OC = OUT // P      # 8 o-chunks of 128

F32 = mybir.dt.float32
F16 = mybir.dt.float16
AF = mybir.ActivationFunctionType

EPS_BUFS = int(os.environ.get("BAYES_EPS_BUFS", "12"))
PROD_BUFS = int(os.environ.get("BAYES_PROD_BUFS", "4"))
DUAL_Q = os.environ.get("BAYES_DUAL_Q", "0") == "1"
SIGREP = os.environ.get("BAYES_SIGREP", "1") == "1"
INPLACE = os.environ.get("BAYES_INPLACE", "1") == "1"
# timing probes (correctness-breaking, never set in the graded path)
NO_MM = os.environ.get("BAYES_NO_MM", "0") == "1"      # skip matvec matmuls
NO_PROD = os.environ.get("BAYES_NO_PROD", "0") == "1"  # matvec on raw ep


def build_nc(repeat: int = 1) -> bass.Bass:
    nc = bacc.Bacc(
        "TRN2",
        target_bir_lowering=False,
        debug=False,
        num_devices=NCORES,
    )

    xT_d = nc.dram_tensor("xT", [P, B], F16, kind="ExternalInput")
    mu_d = nc.dram_tensor("mu", [P, OUT], F16, kind="ExternalInput")
    ro_d = nc.dram_tensor("ro", [P, OUT], F16, kind="ExternalInput")
    mubT_d = nc.dram_tensor("mu_bias_T", [P, OC], F32, kind="ExternalInput")
    robT_d = nc.dram_tensor("ro_bias_T", [P, OC], F32, kind="ExternalInput")
    # chunk-major host layout: per chunk each partition's CB rows are one
    # contiguous 2*CB KB run -> full-rate DMA descriptors
    eps_d = nc.dram_tensor("eps", [CHUNKS, P, CB * OUT], F16, kind="ExternalInput")
    ebsT_d = nc.dram_tensor("eps_bias_T", [P, OC * B], F16, kind="ExternalInput")
    mskb_d = nc.dram_tensor("bmask", [1, B], F32, kind="ExternalInput")
    out_d = nc.dram_tensor("out", [P, OC * B], F16, kind="ExternalOutput")

    with tile.TileContext(nc) as tc:
        with (
            tc.tile_pool(name="const", bufs=1) as const,
            tc.tile_pool(name="sigp", bufs=2) as sigp,
            tc.tile_pool(name="stream", bufs=EPS_BUFS) as stream,
            tc.tile_pool(name="prods", bufs=PROD_BUFS) as prods,
            tc.tile_pool(name="psum_acc", bufs=2, space="PSUM") as psum_acc,
            tc.tile_pool(name="psum_misc", bufs=2, space="PSUM") as psum_misc,
        ):
          with tc.For_i(0, repeat, 1) if repeat > 1 else contextlib.nullcontext():
            # ---------- setup DMAs (Pool queue: never gates prefetch) ----------
            xT_sb = const.tile([P, B], F16, name="xT_sb")
            nc.gpsimd.dma_start(xT_sb, xT_d[:])
            ro_sb = sigp.tile([P, OUT], F16, name="ro_sb")
            nc.gpsimd.dma_start(ro_sb, ro_d[:])
            mu_sb = const.tile([P, OUT], F16, name="mu_sb")
            nc.gpsimd.dma_start(mu_sb, mu_d[:])
            ebsT = const.tile([P, OC, B], F16, name="ebsT")
            nc.gpsimd.dma_start(ebsT, ebsT_d[:].rearrange("p (c b) -> p c b", b=B))
            sbbT = const.tile([P, OC], F32, name="sbbT")
            nc.gpsimd.dma_start(sbbT, robT_d[:])
            mubT = const.tile([P, OC], F32, name="mubT")
            nc.gpsimd.dma_start(mubT, mubT_d[:])
            mskb = const.tile([P, B], F32, name="mskb")
            nc.gpsimd.dma_start(mskb, mskb_d[:].to_broadcast((P, B)))

            # ---------- ACT: sigma (double-buffered, replicated) ----------
            sig_t = sigp.tile([P, OUT], F32, name="sig_t")
            nc.scalar.activation(sig_t, ro_sb, AF.Exp)
            if SIGREP:
                sigr = sigp.tile([P, CB, OUT], F16, name="sigr")
                nc.scalar.activation(sigr[:, 0, :], sig_t, AF.Ln, bias=1.0)
                for t in range(1, CB):
                    nc.scalar.copy(sigr[:, t, :], sigr[:, 0, :])
            else:
                sig = sigp.tile([P, OUT], F16, name="sig")
                nc.scalar.activation(sig, sig_t, AF.Ln, bias=1.0)
            nc.scalar.activation(sbbT, sbbT, AF.Exp)
            nc.scalar.activation(sbbT, sbbT, AF.Ln, bias=1.0)

            # ---------- PE head: xmu partial, ACT evacuates it ----------
            xmu_ps = psum_misc.tile([P, OC, B], F32, name="xmu_ps", tag="xmu")
            for c in range(OC):
                nc.tensor.matmul(
                    xmu_ps[:, c, :], mu_sb[:, ts(c, P)], xT_sb,
                    start=True, stop=True,
                )
            xmu_sb = const.tile([P, OC, B], F32, name="xmu_sb")
            nc.scalar.copy(xmu_sb, xmu_ps)

            # ---------- streaming main loop ----------
            xps = psum_acc.tile([P, OC, B], F32, name="xps", tag="xps")
            if NO_MM:
                nc.vector.memset(xps, 0.0)
            for c in range(CHUNKS):
                ep = stream.tile([P, CB * OUT], F16, name="ep", tag="ep")
                q = nc.scalar if (DUAL_Q and c % 2) else nc.sync
                q.dma_start(ep, eps_d[c])
                if INPLACE:
                    prod = ep
                else:
                    prod = prods.tile([P, CB * OUT], F16, name="prod", tag="prod")
                if not NO_PROD:
                    if SIGREP:
                        nc.vector.tensor_tensor(
                            prod, ep, sigr[:].rearrange("p t o -> p (t o)"),
                            mybir.AluOpType.mult,
                        )
                    else:
                        for t in range(CB):
                            nc.vector.tensor_tensor(
                                prod[:, ts(t, OUT)], ep[:, ts(t, OUT)], sig,
                                mybir.AluOpType.mult,
                            )
                src = ep if NO_PROD else prod
                if not NO_MM:
                    for t in range(CB):
                        b = c * CB + t
                        for oc in range(OC):
                            o0 = t * OUT + oc * P
                            nc.tensor.matmul(
                                xps[:, oc, b : b + 1],
                                src[:, o0 : o0 + P],
                                xT_sb[:, b : b + 1],
                                start=True, stop=True,
                            )

            # ---------- bias chain (DVE, after all prods in queue) ----------
            combT = const.tile([P, OC, B], F32, name="combT")
            for c in range(OC):
                # comb = ebs_T * sigma_b + mu_bias (fused per-partition scalars)
                nc.vector.tensor_scalar(
                    combT[:, c, :], ebsT[:, c, :],
                    sbbT[:, c : c + 1], mubT[:, c : c + 1],
                    op0=mybir.AluOpType.mult, op1=mybir.AluOpType.add,
                )
                nc.vector.tensor_tensor(
                    combT[:, c, :], combT[:, c, :], mskb, mybir.AluOpType.mult
                )
            nc.vector.tensor_add(combT, combT, xmu_sb)
            comb16 = const.tile([P, OC, B], F16, name="comb16")
            nc.vector.tensor_copy(comb16, combT)

            # ---------- epilogue: DVE evac + add, out via DVE queue ----------
            stage = const.tile([P, OC, B], F16, name="stage")
            nc.vector.tensor_copy(stage, xps)
            nc.vector.tensor_add(stage, stage, comb16)
            nc.scalar.dma_start(out_d[:], stage[:].rearrange("p c b -> p (c b)"))

    nc.finalize()
    return nc


def _shard_inputs(inputs: dict) -> list[dict]:
    x = np.asarray(inputs["x"], dtype=np.float32)
    mu = np.asarray(inputs["mu"], dtype=np.float32)
    ro = np.asarray(inputs["ro"], dtype=np.float32)
    mub = np.asarray(inputs["mu_bias"], dtype=np.float32)
    rob = np.asarray(inputs["ro_bias"], dtype=np.float32)
    eps = np.asarray(inputs["eps"], dtype=np.float32)
    ebd = np.asarray(inputs["eps_bias"], dtype=np.float32)

    xT16 = np.ascontiguousarray(x.T.astype(np.float16))       # [IN, B]
    mu16 = mu.astype(np.float16)                              # [IN, OUT]
    ro16 = ro.astype(np.float16)                              # [IN, OUT]
    eps16 = eps.astype(np.float16)                            # [B, IN, OUT]
    # chunk-major: [CHUNKS, IN, CB*OUT]; b = c*CB + t
    eps16 = np.ascontiguousarray(
        eps16.reshape(CHUNKS, CB, IN, OUT)
        .transpose(0, 2, 1, 3)
        .reshape(CHUNKS, IN, CB * OUT)
    )
    # transposed bias operands: [P(o_p), OC(oc), ...] with o = oc*128 + o_p
    ebsT = np.ascontiguousarray(
        ebd.T.reshape(OC, P, B).transpose(1, 0, 2).reshape(P, OC * B)
    ).astype(np.float16)                                      # [128, 8*64]
    mubT = np.ascontiguousarray(mub.reshape(OC, P).T).astype(np.float32)
    robT = np.ascontiguousarray(rob.reshape(OC, P).T).astype(np.float32)

    in_maps = []
    for k in range(NCORES):
        sl = slice(k * P, (k + 1) * P)
        msk = np.zeros((1, B), dtype=np.float32)
        msk[0, k * BL : (k + 1) * BL] = 1.0
        in_maps.append(
            {
                "xT": np.ascontiguousarray(xT16[sl]),
                "mu": np.ascontiguousarray(mu16[sl]),
                "ro": np.ascontiguousarray(ro16[sl]),
                "mu_bias_T": mubT,
                "ro_bias_T": robT,
                "eps": np.ascontiguousarray(eps16[:, sl, :]),  # [CHUNKS, P, CB*OUT]
                "eps_bias_T": ebsT,
                "bmask": msk,
            }
        )
    return in_maps


def _gather(stacked: np.ndarray) -> np.ndarray:
    """[NCORES, P, OC*B] per-core transposed partials -> [B, OUT] f32."""
    a = stacked.reshape(NCORES, P, OC, B).astype(np.float32).sum(axis=0)
    # a[o_p, oc, b] -> out[b, oc*128 + o_p]
    return np.ascontiguousarray(a.transpose(2, 1, 0).reshape(B, OUT))


def run(inputs: dict, trace: bool = False):
    nc = build_nc()
    in_maps = _shard_inputs(inputs)
    res = bass_utils.run_bass_kernel_spmd(
        nc, in_maps, core_ids=list(range(NCORES)), trace=trace
    )
    out = _gather(
        np.stack([res.results[k]["out"] for k in range(NCORES)], axis=0)
    )
    return out, res


def kernel(**inputs: np.ndarray) -> np.ndarray:
    try:
        out, _ = run(inputs, trace=False)
    except Exception:
        # transient device errors (NRT_EXEC_UNIT_UNRECOVERABLE) have been
        # observed to clear on retry
        import time

        time.sleep(5.0)
        out, _ = run(inputs, trace=False)
    return out


# revision 20
# speedup vs baseline: 1.7345x; 1.3096x over previous
"""Trainium2 Bass kernel for nn_BayesianLayer (sampling, contraction-sharded).

Reference computation (per full inputs):
    sigma      = softplus(ro)                  # [IN, OUT]
    sigma_b    = softplus(ro_bias)             # [1, OUT]
    weights    = eps * sigma + mu              # [B, IN, OUT]
    bias       = eps_bias * sigma_b + mu_bias  # [B, OUT]
    out        = einsum("bi,bio->bo", x, weights) + bias

Sharding: the kernel is DMA-bound (eps alone is 256 MB), so the split
minimizes per-core HBM bytes. IN=1024 is sharded across the 8 cores
(128 contraction rows each): eps, mu, ro, x are sharded along IN — so
mu/ro are NOT replicated (vs. 8 MB/core of replicated mu+ro under batch
sharding). Each core emits a partial over its i-slice; the host sums the
8 partials. The bias term is masked onto the core owning those batch
rows (bmask one-hot block) so the host sum adds it exactly once.

Input precision: everything streams as fp16 (host-side cast — DMA
halves, PE runs at 1 cycle/row vs 4 for fp32). Error is ~5e-4
max-relative vs. the 2e-2 gate.

Layout: everything runs TRANSPOSED, out_T[o, b], so all DMA and engine
work spreads across the full 128 partitions (DMA cost scales with
per-partition bytes; a row-major [1, OUT] result row would serialize on
one partition):
  - matvec per (sample b, o-chunk oc): lhsT = (eps*sigma)[128i, 128o]
    stationary, rhs = x column [128i, 1] moving -> psum[128o, oc, b].
    The whole [OUT, B] partial accumulates in ONE psum bank.
  - xmu partial: lhsT = mu chunk [128i, 128o], rhs = xT [128i, 64] ->
    a second bank; 8 matmuls cover it, emitted at the PE queue head.
  - bias in transposed layout: comb_T[o_p, oc, b] =
    bmask[b] * (eps_bias_T[o, b] * sigma_b[o] + mu_bias[o]); operands
    host-pre-transposed, sigma_b/mu_bias as per-partition scalars of a
    fused tensor_scalar (mult+add), the b-mask via a broadcast multiply.
  - epilogue: DVE evacuates psum -> fp16 stage and adds comb_T; one
    128-partition DMA ships out_T [128, 512] fp16. The host transposes
    the partials back.

Queue/buffer discipline for the For_i steady state:
  - eps chunks stream on the sync queue (optionally alternating with the
    scalar queue, BAYES_DUAL_Q) in chunk-major host layout (full 8KB+
    contiguous per-partition lines).
  - setup loads (xT/ro/mu/bias operands) ride the otherwise-idle Pool
    queue so the next iteration's prefetch is never gated.
  - sig/ro live in a bufs=2 pool and the sigma softplus chain is the
    only late ACT work, so iteration i+1's sigma is ready mid-iteration
    i; ACT also replicates sigma CB-fold so the DVE runs ONE
    tensor_tensor per chunk.
  - the bias DVE chain is emitted AFTER the streaming loop so the
    in-order DVE queue runs prods back-to-back across the boundary; the
    out DMA issues from the DVE queue right after the epilogue add.

build_nc(repeat=N) wraps the whole body in a For_i loop — used only by
the timing harness (test.py); the graded path uses repeat=1.
"""

import contextlib
import os

import numpy as np

import concourse.bass as bass
import concourse.mybir as mybir
import concourse.tile as tile
from concourse import bacc
from concourse.bass import ts
from concourse import bass_utils

B, IN, OUT = 64, 1024, 1024
NCORES = 8
P = 128            # SBUF partitions = per-core i-slice (IN / NCORES)
BL = B // NCORES   # batch rows whose bias this core owns
CB = int(os.environ.get("BAYES_CB", "4"))  # samples per eps DMA chunk
CHUNKS = B // CB
OC = OUT // P      # 8 o-chunks of 128

F32 = mybir.dt.float32
F16 = mybir.dt.float16
AF = mybir.ActivationFunctionType

EPS_BUFS = int(os.environ.get("BAYES_EPS_BUFS", "12"))
PROD_BUFS = int(os.environ.get("BAYES_PROD_BUFS", "4"))
DUAL_Q = os.environ.get("BAYES_DUAL_Q", "0") == "1"
SIGREP = os.environ.get("BAYES_SIGREP", "1") == "1"
INPLACE = os.environ.get("BAYES_INPLACE", "1") == "1"
# every POOL_NTH-th chunk's eps*sigma multiply runs on the idle GPSIMD
# engine instead of DVE (0 = off)
POOL_NTH = int(os.environ.get("BAYES_POOL_NTH", "0"))
# timing probes (correctness-breaking, never set in the graded path)
NO_MM = os.environ.get("BAYES_NO_MM", "0") == "1"      # skip matvec matmuls
NO_PROD = os.environ.get("BAYES_NO_PROD", "0") == "1"  # matvec on raw ep


def build_nc(repeat: int = 1) -> bass.Bass:
    nc = bacc.Bacc(
        "TRN2",
        target_bir_lowering=False,
        debug=False,
        num_devices=NCORES,
    )

    xT_d = nc.dram_tensor("xT", [P, B], F16, kind="ExternalInput")
    mu_d = nc.dram_tensor("mu", [P, OUT], F16, kind="ExternalInput")
    ro_d = nc.dram_tensor("ro", [P, OUT], F16, kind="ExternalInput")
    mubT_d = nc.dram_tensor("mu_bias_T", [P, OC], F32, kind="ExternalInput")
    robT_d = nc.dram_tensor("ro_bias_T", [P, OC], F32, kind="ExternalInput")
    # chunk-major host layout: per chunk each partition's CB rows are one
    # contiguous 2*CB KB run -> full-rate DMA descriptors
    eps_d = nc.dram_tensor("eps", [CHUNKS, P, CB * OUT], F16, kind="ExternalInput")
    ebsT_d = nc.dram_tensor("eps_bias_T", [P, OC * B], F16, kind="ExternalInput")
    mskb_d = nc.dram_tensor("bmask", [1, B], F32, kind="ExternalInput")
    out_d = nc.dram_tensor("out", [P, OC * B], F16, kind="ExternalOutput")

    with tile.TileContext(nc) as tc:
        with (
            tc.tile_pool(name="const", bufs=1) as const,
            tc.tile_pool(name="sigp", bufs=2) as sigp,
            tc.tile_pool(name="stream", bufs=EPS_BUFS) as stream,
            tc.tile_pool(name="prods", bufs=PROD_BUFS) as prods,
            tc.tile_pool(name="psum_acc", bufs=2, space="PSUM") as psum_acc,
            tc.tile_pool(name="psum_misc", bufs=2, space="PSUM") as psum_misc,
        ):
          with tc.For_i(0, repeat, 1) if repeat > 1 else contextlib.nullcontext():
            # ---------- setup DMAs (Pool queue: never gates prefetch) ----------
            xT_sb = const.tile([P, B], F16, name="xT_sb")
            nc.gpsimd.dma_start(xT_sb, xT_d[:])
            ro_sb = sigp.tile([P, OUT], F16, name="ro_sb")
            nc.gpsimd.dma_start(ro_sb, ro_d[:])
            mu_sb = const.tile([P, OUT], F16, name="mu_sb")
            nc.gpsimd.dma_start(mu_sb, mu_d[:])
            ebsT = const.tile([P, OC, B], F16, name="ebsT")
            nc.gpsimd.dma_start(ebsT, ebsT_d[:].rearrange("p (c b) -> p c b", b=B))
            sbbT = const.tile([P, OC], F32, name="sbbT")
            nc.gpsimd.dma_start(sbbT, robT_d[:])
            mubT = const.tile([P, OC], F32, name="mubT")
            nc.gpsimd.dma_start(mubT, mubT_d[:])
            mskb = const.tile([P, B], F32, name="mskb")
            nc.gpsimd.dma_start(mskb, mskb_d[:].to_broadcast((P, B)))

            # ---------- ACT: sigma (double-buffered, replicated) ----------
            sig_t = sigp.tile([P, OUT], F32, name="sig_t")
            nc.scalar.activation(sig_t, ro_sb, AF.Exp)
            if SIGREP:
                sigr = sigp.tile([P, CB, OUT], F16, name="sigr")
                nc.scalar.activation(sigr[:, 0, :], sig_t, AF.Ln, bias=1.0)
                for t in range(1, CB):
                    nc.scalar.copy(sigr[:, t, :], sigr[:, 0, :])
            else:
                sig = sigp.tile([P, OUT], F16, name="sig")
                nc.scalar.activation(sig, sig_t, AF.Ln, bias=1.0)
            nc.scalar.activation(sbbT, sbbT, AF.Exp)
            nc.scalar.activation(sbbT, sbbT, AF.Ln, bias=1.0)

            # ---------- PE head: xmu partial, ACT evacuates it ----------
            xmu_ps = psum_misc.tile([P, OC, B], F32, name="xmu_ps", tag="xmu")
            for c in range(OC):
                nc.tensor.matmul(
                    xmu_ps[:, c, :], mu_sb[:, ts(c, P)], xT_sb,
                    start=True, stop=True,
                )
            xmu_sb = const.tile([P, OC, B], F32, name="xmu_sb")
            nc.scalar.copy(xmu_sb, xmu_ps)

            # ---------- streaming main loop ----------
            xps = psum_acc.tile([P, OC, B], F32, name="xps", tag="xps")
            if NO_MM:
                nc.vector.memset(xps, 0.0)
            for c in range(CHUNKS):
                ep = stream.tile([P, CB * OUT], F16, name="ep", tag="ep")
                q = nc.scalar if (DUAL_Q and c % 2) else nc.sync
                q.dma_start(ep, eps_d[c])
                if INPLACE:
                    prod = ep
                else:
                    prod = prods.tile([P, CB * OUT], F16, name="prod", tag="prod")
                if not NO_PROD:
                    if SIGREP:
                        eng = (
                            nc.gpsimd
                            if POOL_NTH and c % POOL_NTH == POOL_NTH - 1
                            else nc.vector
                        )
                        eng.tensor_tensor(
                            prod, ep, sigr[:].rearrange("p t o -> p (t o)"),
                            mybir.AluOpType.mult,
                        )
                    else:
                        for t in range(CB):
                            nc.vector.tensor_tensor(
                                prod[:, ts(t, OUT)], ep[:, ts(t, OUT)], sig,
                                mybir.AluOpType.mult,
                            )
                src = ep if NO_PROD else prod
                if not NO_MM:
                    for t in range(CB):
                        b = c * CB + t
                        for oc in range(OC):
                            o0 = t * OUT + oc * P
                            nc.tensor.matmul(
                                xps[:, oc, b : b + 1],
                                src[:, o0 : o0 + P],
                                xT_sb[:, b : b + 1],
                                start=True, stop=True,
                            )

            # ---------- bias chain (DVE, after all prods in queue) ----------
            combT = const.tile([P, OC, B], F32, name="combT")
            for c in range(OC):
                # comb = ebs_T * sigma_b + mu_bias (fused per-partition scalars)
                nc.vector.tensor_scalar(
                    combT[:, c, :], ebsT[:, c, :],
                    sbbT[:, c : c + 1], mubT[:, c : c + 1],
                    op0=mybir.AluOpType.mult, op1=mybir.AluOpType.add,
                )
                nc.vector.tensor_tensor(
                    combT[:, c, :], combT[:, c, :], mskb, mybir.AluOpType.mult
                )
            nc.vector.tensor_add(combT, combT, xmu_sb)
            comb16 = const.tile([P, OC, B], F16, name="comb16")
            nc.vector.tensor_copy(comb16, combT)

            # ---------- epilogue: DVE evac + add, out via DVE queue ----------
            stage = const.tile([P, OC, B], F16, name="stage")
            nc.vector.tensor_copy(stage, xps)
            nc.vector.tensor_add(stage, stage, comb16)
            nc.scalar.dma_start(out_d[:], stage[:].rearrange("p c b -> p (c b)"))

    nc.finalize()
    return nc


def _shard_inputs(inputs: dict) -> list[dict]:
    x = np.asarray(inputs["x"], dtype=np.float32)
    mu = np.asarray(inputs["mu"], dtype=np.float32)
    ro = np.asarray(inputs["ro"], dtype=np.float32)
    mub = np.asarray(inputs["mu_bias"], dtype=np.float32)
    rob = np.asarray(inputs["ro_bias"], dtype=np.float32)
    eps = np.asarray(inputs["eps"], dtype=np.float32)
    ebd = np.asarray(inputs["eps_bias"], dtype=np.float32)

    xT16 = np.ascontiguousarray(x.T.astype(np.float16))       # [IN, B]
    mu16 = mu.astype(np.float16)                              # [IN, OUT]
    ro16 = ro.astype(np.float16)                              # [IN, OUT]
    eps16 = eps.astype(np.float16)                            # [B, IN, OUT]
    # chunk-major: [CHUNKS, IN, CB*OUT]; b = c*CB + t
    eps16 = np.ascontiguousarray(
        eps16.reshape(CHUNKS, CB, IN, OUT)
        .transpose(0, 2, 1, 3)
        .reshape(CHUNKS, IN, CB * OUT)
    )
    # transposed bias operands: [P(o_p), OC(oc), ...] with o = oc*128 + o_p
    ebsT = np.ascontiguousarray(
        ebd.T.reshape(OC, P, B).transpose(1, 0, 2).reshape(P, OC * B)
    ).astype(np.float16)                                      # [128, 8*64]
    mubT = np.ascontiguousarray(mub.reshape(OC, P).T).astype(np.float32)
    robT = np.ascontiguousarray(rob.reshape(OC, P).T).astype(np.float32)

    in_maps = []
    for k in range(NCORES):
        sl = slice(k * P, (k + 1) * P)
        msk = np.zeros((1, B), dtype=np.float32)
        msk[0, k * BL : (k + 1) * BL] = 1.0
        in_maps.append(
            {
                "xT": np.ascontiguousarray(xT16[sl]),
                "mu": np.ascontiguousarray(mu16[sl]),
                "ro": np.ascontiguousarray(ro16[sl]),
                "mu_bias_T": mubT,
                "ro_bias_T": robT,
                "eps": np.ascontiguousarray(eps16[:, sl, :]),  # [CHUNKS, P, CB*OUT]
                "eps_bias_T": ebsT,
                "bmask": msk,
            }
        )
    return in_maps


def _gather(stacked: np.ndarray) -> np.ndarray:
    """[NCORES, P, OC*B] per-core transposed partials -> [B, OUT] f32."""
    a = stacked.reshape(NCORES, P, OC, B).astype(np.float32).sum(axis=0)
    # a[o_p, oc, b] -> out[b, oc*128 + o_p]
    return np.ascontiguousarray(a.transpose(2, 1, 0).reshape(B, OUT))


def run(inputs: dict, trace: bool = False):
    nc = build_nc()
    in_maps = _shard_inputs(inputs)
    res = bass_utils.run_bass_kernel_spmd(
        nc, in_maps, core_ids=list(range(NCORES)), trace=trace
    )
    out = _gather(
        np.stack([res.results[k]["out"] for k in range(NCORES)], axis=0)
    )
    return out, res


def kernel(**inputs: np.ndarray) -> np.ndarray:
    try:
        out, _ = run(inputs, trace=False)
    except Exception:
        # transient device errors (NRT_EXEC_UNIT_UNRECOVERABLE) have been
        # observed to clear on retry
        import time

        time.sleep(5.0)
        out, _ = run(inputs, trace=False)
    return out
